# revision 10
# baseline (speedup 1.0000x reference)
"""Trainium2 Bass kernel for nn_AlignLayer (1D RoI-Align, nms_detection).

Formulation: the RoI-Align output is a linear map of the tiny feature map:
    out[k, c, (i, di, s)] = sum_t x[k, c, t] * M[t, (i, di, s)]
where M (the interpolation-weight matrix) depends only on the anchors, which
are data-independent. M is built on the host in float32 (bit-mirroring the
reference's sampling arithmetic), cast to fp16, and streamed to the cores.
Each core computes a di-slice (8 of 64 durations) of the output with PE
matmuls (x^T stationary, M moving), evacuates PSUM via DVE/ACT copies, and
DMAs fp32 results out.  Sharding: data-parallel over the di axis; x is
replicated (it is only 400 KB).
"""

import numpy as np

import concourse.bass as bass  # noqa: F401  (registers engine classes)
import concourse.mybir as mybir
import concourse.tile as tile
from concourse import bacc
from concourse.bass_utils import run_bass_kernel_spmd

BS, T, C, RES, DUR = 4, 200, 128, 16, 64
MAX_N = 8
N_CORES = 8
DI_PER = DUR // N_CORES          # 8 durations per core
NCOL = RES * DI_PER * T          # 25600 output columns per (batch, core)
SG = 5120                        # staging block (columns)
NSG = NCOL // SG                 # 5
GRP = 1024                       # psum group (2 banks)
NGRP = SG // GRP                 # 5
MMN = 512                        # matmul free dim (1 psum bank of fp32)
KA, KB = 128, T - 128            # contraction split 128 + 72

_CACHE = {}


def _build_weight_matrix(anchors: np.ndarray) -> np.ndarray:
    """M[t, i, di, s] in float64, mirroring reference.align1d float32 math."""
    f32 = np.float32
    A = np.asarray(anchors, dtype=np.float32)
    Rk = T * DUR
    a0 = A[:Rk]
    s_ = a0[:, 1]
    e_ = a0[:, 2]
    roi = np.maximum(e_ - s_, f32(1.0))
    binsz = roi / f32(RES)
    n = np.ceil(roi / f32(RES))
    ii = np.arange(RES, dtype=np.float32)
    ii_int = np.arange(RES, dtype=np.int64)

    di_arr = np.arange(Rk, dtype=np.int64) % DUR
    s_arr = np.arange(Rk, dtype=np.int64) // DUR
    col_part = di_arr * T + s_arr                       # [Rk]

    nflat = T * RES * DUR * T
    Macc = np.zeros(nflat, dtype=np.float64)
    for j in range(MAX_N):
        y = s_[:, None] + ii[None, :] * binsz[:, None] \
            + f32(j + 0.5) * binsz[:, None] / n[:, None]          # [Rk, RES]
        valid = (y >= f32(-1.0)) & (y <= f32(T)) & (f32(j) < n[:, None])
        yc = np.maximum(y, f32(0.0))
        ylof = np.floor(yc)
        at_end = ylof >= f32(T - 1)
        ylo = np.clip(ylof.astype(np.int64), 0, T - 1)
        yhi = np.clip(ylo + 1, 0, T - 1)
        ly = np.where(at_end, f32(0.0), yc - ylof).astype(np.float32)
        wlo = ((f32(1.0) - ly) / n[:, None]).astype(np.float64)
        whi = (ly / n[:, None]).astype(np.float64)
        flat_lo = (ylo * RES + ii_int[None, :]) * (DUR * T) + col_part[:, None]
        flat_hi = (yhi * RES + ii_int[None, :]) * (DUR * T) + col_part[:, None]
        v = valid.ravel()
        Macc += np.bincount(flat_lo.ravel()[v], weights=wlo.ravel()[v],
                            minlength=nflat)
        Macc += np.bincount(flat_hi.ravel()[v], weights=whi.ravel()[v],
                            minlength=nflat)
    return Macc.reshape(T, RES, DUR, T)


def _build_nc():
    nc = bacc.Bacc("TRN2", target_bir_lowering=False, debug=False)
    xt = nc.dram_tensor("xt", [BS, T, C], mybir.dt.float16, kind="ExternalInput")
    mh = nc.dram_tensor("mh", [KA, NCOL], mybir.dt.float16, kind="ExternalInput")
    ml = nc.dram_tensor("ml", [KB, NCOL], mybir.dt.float16, kind="ExternalInput")
    out = nc.dram_tensor("out", [BS, C, NCOL], mybir.dt.float32,
                         kind="ExternalOutput")
    with tile.TileContext(nc) as tc:
        with (
            tc.tile_pool(name="xp", bufs=1) as xp,
            tc.tile_pool(name="mp", bufs=3) as mp,
            tc.tile_pool(name="stp", bufs=4) as stp,
            tc.tile_pool(name="psp", bufs=4, space="PSUM") as psp,
        ):
            # sg-0 M tiles load FIRST on the ACT ring — they gate the
            # first matmul; the tiny xt loads follow (finish early anyway).
            mtiles = {}
            mht = mp.tile([KA, SG], mybir.dt.float16, tag="mh", name="mh0")
            mlt = mp.tile([KB, SG], mybir.dt.float16, tag="ml", name="ml0")
            nc.scalar.dma_start(mht[:], mh[:, 0:SG])
            nc.scalar.dma_start(mlt[:], ml[:, 0:SG])
            mtiles[0] = (mht, mlt)
            xa, xb = [], []
            for k in range(BS):
                ta = xp.tile([KA, C], mybir.dt.float16, tag=f"xa{k}", name=f"xa{k}")
                tb = xp.tile([KB, C], mybir.dt.float16, tag=f"xb{k}", name=f"xb{k}")
                nc.scalar.dma_start(ta[:], xt[k, 0:KA, :])
                nc.scalar.dma_start(tb[:], xt[k, KA:T, :])
                xa.append(ta)
                xb.append(tb)
            copy_flip = 0
            for sg in range(NSG):
                c0 = sg * SG
                if sg not in mtiles:
                    mht = mp.tile([KA, SG], mybir.dt.float16, tag="mh",
                                  name=f"mh{sg}")
                    mlt = mp.tile([KB, SG], mybir.dt.float16, tag="ml",
                                  name=f"ml{sg}")
                    nc.scalar.dma_start(mht[:], mh[:, c0:c0 + SG])
                    nc.scalar.dma_start(mlt[:], ml[:, c0:c0 + SG])
                else:
                    mht, mlt = mtiles[sg]
                for k in range(BS):
                    stg = stp.tile([C, SG], mybir.dt.float32, tag="stg",
                                   name=f"stg{sg}_{k}")
                    for g in range(NGRP):
                        ps = psp.tile([C, GRP], mybir.dt.float32, tag="ps",
                                      name=f"ps{sg}_{k}_{g}")
                        for h in range(GRP // MMN):
                            lo = g * GRP + h * MMN
                            nc.tensor.matmul(
                                ps[:, h * MMN:(h + 1) * MMN], xa[k],
                                mht[:, lo:lo + MMN], start=True, stop=False)
                        for h in range(GRP // MMN):
                            lo = g * GRP + h * MMN
                            nc.tensor.matmul(
                                ps[:, h * MMN:(h + 1) * MMN], xb[k],
                                mlt[:, lo:lo + MMN], start=False, stop=True)
                        dst = stg[:, g * GRP:(g + 1) * GRP]
                        if copy_flip % 2 == 0:
                            nc.scalar.copy(dst, ps[:])
                        else:
                            nc.vector.tensor_copy(dst, ps[:])
                        copy_flip += 1
                    if sg == 0 and k == 0:
                        # Head edge: first MB leaves as soon as groups 0-1
                        # are evacuated (subtile deps).
                        nc.sync.dma_start(out[k, :, c0:c0 + 2 * GRP],
                                          stg[:, :2 * GRP])
                        nc.sync.dma_start(out[k, :, c0 + 2 * GRP:c0 + SG],
                                          stg[:, 2 * GRP:])
                    elif sg == NSG - 1 and k == BS - 1:
                        # Tail edge: drain in shrinking pieces.
                        nc.sync.dma_start(out[k, :, c0:c0 + 3 * GRP],
                                          stg[:, :3 * GRP])
                        nc.sync.dma_start(out[k, :, c0 + 3 * GRP:c0 + 4 * GRP],
                                          stg[:, 3 * GRP:4 * GRP])
                        nc.sync.dma_start(out[k, :, c0 + 4 * GRP:c0 + SG],
                                          stg[:, 4 * GRP:])
                    else:
                        nc.sync.dma_start(out[k, :, c0:c0 + SG], stg[:])
    nc.compile()
    return nc


def _anchors_batch_uniform(A: np.ndarray) -> bool:
    Rk = T * DUR
    if A.shape != (BS * Rk, 3):
        return False
    A4 = A.reshape(BS, Rk, 3)
    return (np.all(A4[:, :, 1:] == A4[:1, :, 1:])
            and np.all(A4[:, :, 0] == np.arange(BS, dtype=A.dtype)[:, None]))


def _host_fallback(x: np.ndarray, anchors: np.ndarray) -> np.ndarray:
    """Exact numpy replica of the reference path (slow; safety net only)."""
    f32 = np.float32
    A = np.asarray(anchors, dtype=np.float32)
    R = A.shape[0]
    b = A[:, 0].astype(np.int64)
    s_, e_ = A[:, 1], A[:, 2]
    roi = np.maximum(e_ - s_, f32(1.0))
    binsz = roi / f32(RES)
    n = np.ceil(roi / f32(RES))
    ii = np.arange(RES, dtype=np.float32)
    xf = np.asarray(x, dtype=np.float32).transpose(0, 2, 1).reshape(BS * T, C)
    acc = np.zeros((R, RES, C), np.float32)
    for j in range(MAX_N):
        y = s_[:, None] + ii[None, :] * binsz[:, None] \
            + f32(j + 0.5) * binsz[:, None] / n[:, None]
        valid = (y >= f32(-1.0)) & (y <= f32(T)) & (f32(j) < n[:, None])
        yc = np.maximum(y, f32(0.0))
        ylof = np.floor(yc)
        at_end = ylof >= f32(T - 1)
        ylo = np.clip(ylof.astype(np.int64), 0, T - 1)
        yhi = np.clip(ylo + 1, 0, T - 1)
        ly = np.where(at_end, f32(0.0), yc - ylof).astype(np.float32)
        base = b[:, None] * T
        v = (1.0 - ly)[..., None] * xf[base + ylo] + ly[..., None] * xf[base + yhi]
        acc += np.where(valid[..., None], v, 0.0)
    acc /= n[:, None, None]
    feats = acc.transpose(0, 2, 1).reshape(BS, T, DUR, C * RES)
    return np.ascontiguousarray(feats.transpose(0, 3, 2, 1))


def kernel(x: np.ndarray, anchors: np.ndarray) -> np.ndarray:
    x = np.asarray(x)
    anchors = np.asarray(anchors)
    if not _anchors_batch_uniform(np.asarray(anchors, dtype=np.float32)):
        return _host_fallback(x, anchors)
    Mfull = _build_weight_matrix(anchors)               # [T, RES, DUR, T] f64
    xt16 = np.ascontiguousarray(
        x.astype(np.float32).transpose(0, 2, 1)).astype(np.float16)

    if "nc" not in _CACHE:
        _CACHE["nc"] = _build_nc()
    nc = _CACHE["nc"]

    in_maps = []
    for m in range(N_CORES):
        Mc = np.ascontiguousarray(
            Mfull[:, :, m * DI_PER:(m + 1) * DI_PER, :]
        ).reshape(T, NCOL).astype(np.float16)
        in_maps.append({
            "xt": xt16,
            "mh": np.ascontiguousarray(Mc[0:KA]),
            "ml": np.ascontiguousarray(Mc[KA:T]),
        })

    res = run_bass_kernel_spmd(nc, in_maps, core_ids=list(range(N_CORES)))
    full = np.empty((BS, C, RES, DUR, T), dtype=np.float32)
    for m in range(N_CORES):
        full[:, :, :, m * DI_PER:(m + 1) * DI_PER, :] = \
            res.results[m]["out"].reshape(BS, C, RES, DI_PER, T)
    return full.reshape(BS, C * RES, DUR, T)


# revision 11
# speedup vs baseline: 1.0945x; 1.0945x over previous
"""Trainium2 Bass kernel for nn_AlignLayer (1D RoI-Align, nms_detection).

Formulation: the RoI-Align output is a linear map of the tiny feature map:
    out[k, c, (i, di, s)] = sum_t x[k, c, t] * M[t, (i, di, s)]
where M (the interpolation-weight matrix) depends only on the anchors, which
are data-independent. M is built on the host in float32 (bit-mirroring the
reference's sampling arithmetic), cast to fp16, and streamed to the cores.
Each core computes a di-slice (8 of 64 durations) of the output with PE
matmuls (x^T stationary, M moving), evacuates PSUM via DVE/ACT copies, and
DMAs fp32 results out.  Sharding: data-parallel over the di axis; x is
replicated (it is only 400 KB).
"""

import numpy as np

import concourse.bass as bass  # noqa: F401  (registers engine classes)
import concourse.mybir as mybir
import concourse.tile as tile
from concourse import bacc
from concourse.bass_utils import run_bass_kernel_spmd

BS, T, C, RES, DUR = 4, 200, 128, 16, 64
MAX_N = 8
N_CORES = 8
DI_PER = DUR // N_CORES          # 8 durations per core
NCOL = RES * DI_PER * T          # 25600 output columns per (batch, core)
SG = 5120                        # staging block (columns)
NSG = NCOL // SG                 # 5
GRP = 1024                       # psum group (2 banks)
NGRP = SG // GRP                 # 5
MMN = 512                        # matmul free dim (1 psum bank of fp32)
KA, KB = 128, T - 128            # contraction split 128 + 72

_CACHE = {}


def _build_weight_matrix(anchors: np.ndarray) -> np.ndarray:
    """M[t, i, di, s] in float64, mirroring reference.align1d float32 math."""
    f32 = np.float32
    A = np.asarray(anchors, dtype=np.float32)
    Rk = T * DUR
    a0 = A[:Rk]
    s_ = a0[:, 1]
    e_ = a0[:, 2]
    roi = np.maximum(e_ - s_, f32(1.0))
    binsz = roi / f32(RES)
    n = np.ceil(roi / f32(RES))
    ii = np.arange(RES, dtype=np.float32)
    ii_int = np.arange(RES, dtype=np.int64)

    di_arr = np.arange(Rk, dtype=np.int64) % DUR
    s_arr = np.arange(Rk, dtype=np.int64) // DUR
    col_part = di_arr * T + s_arr                       # [Rk]

    nflat = T * RES * DUR * T
    Macc = np.zeros(nflat, dtype=np.float64)
    for j in range(MAX_N):
        y = s_[:, None] + ii[None, :] * binsz[:, None] \
            + f32(j + 0.5) * binsz[:, None] / n[:, None]          # [Rk, RES]
        valid = (y >= f32(-1.0)) & (y <= f32(T)) & (f32(j) < n[:, None])
        yc = np.maximum(y, f32(0.0))
        ylof = np.floor(yc)
        at_end = ylof >= f32(T - 1)
        ylo = np.clip(ylof.astype(np.int64), 0, T - 1)
        yhi = np.clip(ylo + 1, 0, T - 1)
        ly = np.where(at_end, f32(0.0), yc - ylof).astype(np.float32)
        wlo = ((f32(1.0) - ly) / n[:, None]).astype(np.float64)
        whi = (ly / n[:, None]).astype(np.float64)
        flat_lo = (ylo * RES + ii_int[None, :]) * (DUR * T) + col_part[:, None]
        flat_hi = (yhi * RES + ii_int[None, :]) * (DUR * T) + col_part[:, None]
        v = valid.ravel()
        Macc += np.bincount(flat_lo.ravel()[v], weights=wlo.ravel()[v],
                            minlength=nflat)
        Macc += np.bincount(flat_hi.ravel()[v], weights=whi.ravel()[v],
                            minlength=nflat)
    return Macc.reshape(T, RES, DUR, T)


def _build_nc():
    nc = bacc.Bacc("TRN2", target_bir_lowering=False, debug=False)
    xt = nc.dram_tensor("xt", [BS, T, C], mybir.dt.float16, kind="ExternalInput")
    mh = nc.dram_tensor("mh", [KA, NCOL], mybir.dt.float16, kind="ExternalInput")
    ml = nc.dram_tensor("ml", [KB, NCOL], mybir.dt.float16, kind="ExternalInput")
    out = nc.dram_tensor("out", [BS, C, NCOL], mybir.dt.float32,
                         kind="ExternalOutput")
    with tile.TileContext(nc) as tc:
        with (
            tc.tile_pool(name="xp", bufs=1) as xp,
            tc.tile_pool(name="mp", bufs=3) as mp,
            tc.tile_pool(name="stp", bufs=4) as stp,
            tc.tile_pool(name="psp", bufs=4, space="PSUM") as psp,
        ):
            # sg-0 M tiles load FIRST on the ACT ring — they gate the
            # first matmul; the tiny xt loads follow (finish early anyway).
            mtiles = {}
            mht = mp.tile([KA, SG], mybir.dt.float16, tag="mh", name="mh0")
            mlt = mp.tile([KB, SG], mybir.dt.float16, tag="ml", name="ml0")
            nc.scalar.dma_start(mht[:], mh[:, 0:SG])
            nc.scalar.dma_start(mlt[:], ml[:, 0:SG])
            mtiles[0] = (mht, mlt)
            xa, xb = [], []
            for k in range(BS):
                ta = xp.tile([KA, C], mybir.dt.float16, tag=f"xa{k}", name=f"xa{k}")
                tb = xp.tile([KB, C], mybir.dt.float16, tag=f"xb{k}", name=f"xb{k}")
                nc.scalar.dma_start(ta[:], xt[k, 0:KA, :])
                nc.scalar.dma_start(tb[:], xt[k, KA:T, :])
                xa.append(ta)
                xb.append(tb)
            copy_flip = 0
            for sg in range(NSG):
                c0 = sg * SG
                if sg not in mtiles:
                    mht = mp.tile([KA, SG], mybir.dt.float16, tag="mh",
                                  name=f"mh{sg}")
                    mlt = mp.tile([KB, SG], mybir.dt.float16, tag="ml",
                                  name=f"ml{sg}")
                    nc.scalar.dma_start(mht[:], mh[:, c0:c0 + SG])
                    nc.scalar.dma_start(mlt[:], ml[:, c0:c0 + SG])
                else:
                    mht, mlt = mtiles[sg]
                for k in range(BS):
                    stg = stp.tile([C, SG], mybir.dt.float32, tag="stg",
                                   name=f"stg{sg}_{k}")
                    for g in range(NGRP):
                        ps = psp.tile([C, GRP], mybir.dt.float32, tag="ps",
                                      name=f"ps{sg}_{k}_{g}")
                        for h in range(GRP // MMN):
                            lo = g * GRP + h * MMN
                            nc.tensor.matmul(
                                ps[:, h * MMN:(h + 1) * MMN], xa[k],
                                mht[:, lo:lo + MMN], start=True, stop=False)
                        for h in range(GRP // MMN):
                            lo = g * GRP + h * MMN
                            nc.tensor.matmul(
                                ps[:, h * MMN:(h + 1) * MMN], xb[k],
                                mlt[:, lo:lo + MMN], start=False, stop=True)
                        dst = stg[:, g * GRP:(g + 1) * GRP]
                        if copy_flip % 2 == 0:
                            nc.scalar.copy(dst, ps[:])
                        else:
                            nc.vector.tensor_copy(dst, ps[:])
                        copy_flip += 1
                    if sg == 0 and k == 0:
                        # Head edge: first MB leaves as soon as groups 0-1
                        # are evacuated (subtile deps).
                        nc.sync.dma_start(out[k, :, c0:c0 + 2 * GRP],
                                          stg[:, :2 * GRP])
                        nc.sync.dma_start(out[k, :, c0 + 2 * GRP:c0 + SG],
                                          stg[:, 2 * GRP:])
                    elif sg == NSG - 1 and k == BS - 1:
                        # Tail edge: drain in two pieces.
                        nc.sync.dma_start(out[k, :, c0:c0 + 3 * GRP],
                                          stg[:, :3 * GRP])
                        nc.sync.dma_start(out[k, :, c0 + 3 * GRP:c0 + SG],
                                          stg[:, 3 * GRP:])
                    else:
                        nc.sync.dma_start(out[k, :, c0:c0 + SG], stg[:])
    nc.compile()
    return nc


def _anchors_batch_uniform(A: np.ndarray) -> bool:
    Rk = T * DUR
    if A.shape != (BS * Rk, 3):
        return False
    A4 = A.reshape(BS, Rk, 3)
    return (np.all(A4[:, :, 1:] == A4[:1, :, 1:])
            and np.all(A4[:, :, 0] == np.arange(BS, dtype=A.dtype)[:, None]))


def _host_fallback(x: np.ndarray, anchors: np.ndarray) -> np.ndarray:
    """Exact numpy replica of the reference path (slow; safety net only)."""
    f32 = np.float32
    A = np.asarray(anchors, dtype=np.float32)
    R = A.shape[0]
    b = A[:, 0].astype(np.int64)
    s_, e_ = A[:, 1], A[:, 2]
    roi = np.maximum(e_ - s_, f32(1.0))
    binsz = roi / f32(RES)
    n = np.ceil(roi / f32(RES))
    ii = np.arange(RES, dtype=np.float32)
    xf = np.asarray(x, dtype=np.float32).transpose(0, 2, 1).reshape(BS * T, C)
    acc = np.zeros((R, RES, C), np.float32)
    for j in range(MAX_N):
        y = s_[:, None] + ii[None, :] * binsz[:, None] \
            + f32(j + 0.5) * binsz[:, None] / n[:, None]
        valid = (y >= f32(-1.0)) & (y <= f32(T)) & (f32(j) < n[:, None])
        yc = np.maximum(y, f32(0.0))
        ylof = np.floor(yc)
        at_end = ylof >= f32(T - 1)
        ylo = np.clip(ylof.astype(np.int64), 0, T - 1)
        yhi = np.clip(ylo + 1, 0, T - 1)
        ly = np.where(at_end, f32(0.0), yc - ylof).astype(np.float32)
        base = b[:, None] * T
        v = (1.0 - ly)[..., None] * xf[base + ylo] + ly[..., None] * xf[base + yhi]
        acc += np.where(valid[..., None], v, 0.0)
    acc /= n[:, None, None]
    feats = acc.transpose(0, 2, 1).reshape(BS, T, DUR, C * RES)
    return np.ascontiguousarray(feats.transpose(0, 3, 2, 1))


def kernel(x: np.ndarray, anchors: np.ndarray) -> np.ndarray:
    x = np.asarray(x)
    anchors = np.asarray(anchors)
    if not _anchors_batch_uniform(np.asarray(anchors, dtype=np.float32)):
        return _host_fallback(x, anchors)
    Mfull = _build_weight_matrix(anchors)               # [T, RES, DUR, T] f64
    xt16 = np.ascontiguousarray(
        x.astype(np.float32).transpose(0, 2, 1)).astype(np.float16)

    if "nc" not in _CACHE:
        _CACHE["nc"] = _build_nc()
    nc = _CACHE["nc"]

    in_maps = []
    for m in range(N_CORES):
        Mc = np.ascontiguousarray(
            Mfull[:, :, m * DI_PER:(m + 1) * DI_PER, :]
        ).reshape(T, NCOL).astype(np.float16)
        in_maps.append({
            "xt": xt16,
            "mh": np.ascontiguousarray(Mc[0:KA]),
            "ml": np.ascontiguousarray(Mc[KA:T]),
        })

    res = run_bass_kernel_spmd(nc, in_maps, core_ids=list(range(N_CORES)))
    full = np.empty((BS, C, RES, DUR, T), dtype=np.float32)
    for m in range(N_CORES):
        full[:, :, :, m * DI_PER:(m + 1) * DI_PER, :] = \
            res.results[m]["out"].reshape(BS, C, RES, DI_PER, T)
    return full.reshape(BS, C * RES, DUR, T)


# revision 12
# speedup vs baseline: 1.0994x; 1.0044x over previous
"""Trainium2 Bass kernel for nn_AlignLayer (1D RoI-Align, nms_detection).

Formulation: the RoI-Align output is a linear map of the tiny feature map:
    out[k, c, (i, di, s)] = sum_t x[k, c, t] * M[t, (i, di, s)]
where M (the interpolation-weight matrix) depends only on the anchors, which
are data-independent. M is built on the host in float32 (bit-mirroring the
reference's sampling arithmetic), cast to fp16, and streamed to the cores.
Each core computes a di-slice (8 of 64 durations) of the output with PE
matmuls (x^T stationary, M moving), evacuates PSUM via DVE/ACT copies, and
DMAs fp32 results out.  Sharding: data-parallel over the di axis; x is
replicated (it is only 400 KB).
"""

import numpy as np

import concourse.bass as bass  # noqa: F401  (registers engine classes)
import concourse.mybir as mybir
import concourse.tile as tile
from concourse import bacc
from concourse.bass_utils import run_bass_kernel_spmd

BS, T, C, RES, DUR = 4, 200, 128, 16, 64
MAX_N = 8
N_CORES = 8
DI_PER = DUR // N_CORES          # 8 durations per core
NCOL = RES * DI_PER * T          # 25600 output columns per (batch, core)
SG = 5120                        # staging block (columns)
NSG = NCOL // SG                 # 5
GRP = 1024                       # psum group (2 banks)
NGRP = SG // GRP                 # 5
MMN = 512                        # matmul free dim (1 psum bank of fp32)
KA, KB = 128, T - 128            # contraction split 128 + 72

_CACHE = {}


def _build_weight_matrix(anchors: np.ndarray) -> np.ndarray:
    """M[t, i, di, s] in float64, mirroring reference.align1d float32 math."""
    f32 = np.float32
    A = np.asarray(anchors, dtype=np.float32)
    Rk = T * DUR
    a0 = A[:Rk]
    s_ = a0[:, 1]
    e_ = a0[:, 2]
    roi = np.maximum(e_ - s_, f32(1.0))
    binsz = roi / f32(RES)
    n = np.ceil(roi / f32(RES))
    ii = np.arange(RES, dtype=np.float32)
    ii_int = np.arange(RES, dtype=np.int64)

    di_arr = np.arange(Rk, dtype=np.int64) % DUR
    s_arr = np.arange(Rk, dtype=np.int64) // DUR
    col_part = di_arr * T + s_arr                       # [Rk]

    nflat = T * RES * DUR * T
    Macc = np.zeros(nflat, dtype=np.float64)
    for j in range(MAX_N):
        y = s_[:, None] + ii[None, :] * binsz[:, None] \
            + f32(j + 0.5) * binsz[:, None] / n[:, None]          # [Rk, RES]
        valid = (y >= f32(-1.0)) & (y <= f32(T)) & (f32(j) < n[:, None])
        yc = np.maximum(y, f32(0.0))
        ylof = np.floor(yc)
        at_end = ylof >= f32(T - 1)
        ylo = np.clip(ylof.astype(np.int64), 0, T - 1)
        yhi = np.clip(ylo + 1, 0, T - 1)
        ly = np.where(at_end, f32(0.0), yc - ylof).astype(np.float32)
        wlo = ((f32(1.0) - ly) / n[:, None]).astype(np.float64)
        whi = (ly / n[:, None]).astype(np.float64)
        flat_lo = (ylo * RES + ii_int[None, :]) * (DUR * T) + col_part[:, None]
        flat_hi = (yhi * RES + ii_int[None, :]) * (DUR * T) + col_part[:, None]
        v = valid.ravel()
        Macc += np.bincount(flat_lo.ravel()[v], weights=wlo.ravel()[v],
                            minlength=nflat)
        Macc += np.bincount(flat_hi.ravel()[v], weights=whi.ravel()[v],
                            minlength=nflat)
    return Macc.reshape(T, RES, DUR, T)


def _build_nc():
    nc = bacc.Bacc("TRN2", target_bir_lowering=False, debug=False)
    xt = nc.dram_tensor("xt", [BS, T, C], mybir.dt.float16, kind="ExternalInput")
    mh = nc.dram_tensor("mh", [KA, NCOL], mybir.dt.float16, kind="ExternalInput")
    ml = nc.dram_tensor("ml", [KB, NCOL], mybir.dt.float16, kind="ExternalInput")
    out = nc.dram_tensor("out", [BS, C, NCOL], mybir.dt.float32,
                         kind="ExternalOutput")
    with tile.TileContext(nc) as tc:
        with (
            tc.tile_pool(name="xp", bufs=1) as xp,
            tc.tile_pool(name="mp", bufs=5) as mp,
            tc.tile_pool(name="stp", bufs=3) as stp,
            tc.tile_pool(name="psp", bufs=4, space="PSUM") as psp,
        ):
            # sg-0 M tiles load FIRST on the ACT ring — they gate the
            # first matmul; the tiny xt loads follow (finish early anyway).
            mtiles = {}
            mht = mp.tile([KA, SG], mybir.dt.float16, tag="mh", name="mh0")
            mlt = mp.tile([KB, SG], mybir.dt.float16, tag="ml", name="ml0")
            nc.scalar.dma_start(mht[:], mh[:, 0:SG])
            nc.scalar.dma_start(mlt[:], ml[:, 0:SG])
            mtiles[0] = (mht, mlt)
            xa, xb = [], []
            for k in range(BS):
                ta = xp.tile([KA, C], mybir.dt.float16, tag=f"xa{k}", name=f"xa{k}")
                tb = xp.tile([KB, C], mybir.dt.float16, tag=f"xb{k}", name=f"xb{k}")
                nc.scalar.dma_start(ta[:], xt[k, 0:KA, :])
                nc.scalar.dma_start(tb[:], xt[k, KA:T, :])
                xa.append(ta)
                xb.append(tb)
            for sg in range(1, NSG):
                c0 = sg * SG
                mht = mp.tile([KA, SG], mybir.dt.float16, tag="mh",
                              name=f"mh{sg}")
                mlt = mp.tile([KB, SG], mybir.dt.float16, tag="ml",
                              name=f"ml{sg}")
                nc.scalar.dma_start(mht[:], mh[:, c0:c0 + SG])
                nc.scalar.dma_start(mlt[:], ml[:, c0:c0 + SG])
                mtiles[sg] = (mht, mlt)
            copy_flip = 0
            for sg in range(NSG):
                c0 = sg * SG
                mht, mlt = mtiles[sg]
                for k in range(BS):
                    stg = stp.tile([C, SG], mybir.dt.float32, tag="stg",
                                   name=f"stg{sg}_{k}")
                    for g in range(NGRP):
                        ps = psp.tile([C, GRP], mybir.dt.float32, tag="ps",
                                      name=f"ps{sg}_{k}_{g}")
                        for h in range(GRP // MMN):
                            lo = g * GRP + h * MMN
                            nc.tensor.matmul(
                                ps[:, h * MMN:(h + 1) * MMN], xa[k],
                                mht[:, lo:lo + MMN], start=True, stop=False)
                        for h in range(GRP // MMN):
                            lo = g * GRP + h * MMN
                            nc.tensor.matmul(
                                ps[:, h * MMN:(h + 1) * MMN], xb[k],
                                mlt[:, lo:lo + MMN], start=False, stop=True)
                        dst = stg[:, g * GRP:(g + 1) * GRP]
                        if copy_flip % 2 == 0:
                            nc.scalar.copy(dst, ps[:])
                        else:
                            nc.vector.tensor_copy(dst, ps[:])
                        copy_flip += 1
                    if sg == 0 and k == 0:
                        # Head edge: first MB leaves as soon as groups 0-1
                        # are evacuated (subtile deps).
                        nc.sync.dma_start(out[k, :, c0:c0 + 2 * GRP],
                                          stg[:, :2 * GRP])
                        nc.sync.dma_start(out[k, :, c0 + 2 * GRP:c0 + SG],
                                          stg[:, 2 * GRP:])
                    elif sg == NSG - 1 and k == BS - 1:
                        # Tail edge: drain in two pieces.
                        nc.sync.dma_start(out[k, :, c0:c0 + 3 * GRP],
                                          stg[:, :3 * GRP])
                        nc.sync.dma_start(out[k, :, c0 + 3 * GRP:c0 + SG],
                                          stg[:, 3 * GRP:])
                    else:
                        nc.sync.dma_start(out[k, :, c0:c0 + SG], stg[:])
    nc.compile()
    return nc


def _anchors_batch_uniform(A: np.ndarray) -> bool:
    Rk = T * DUR
    if A.shape != (BS * Rk, 3):
        return False
    A4 = A.reshape(BS, Rk, 3)
    return (np.all(A4[:, :, 1:] == A4[:1, :, 1:])
            and np.all(A4[:, :, 0] == np.arange(BS, dtype=A.dtype)[:, None]))


def _host_fallback(x: np.ndarray, anchors: np.ndarray) -> np.ndarray:
    """Exact numpy replica of the reference path (slow; safety net only)."""
    f32 = np.float32
    A = np.asarray(anchors, dtype=np.float32)
    R = A.shape[0]
    b = A[:, 0].astype(np.int64)
    s_, e_ = A[:, 1], A[:, 2]
    roi = np.maximum(e_ - s_, f32(1.0))
    binsz = roi / f32(RES)
    n = np.ceil(roi / f32(RES))
    ii = np.arange(RES, dtype=np.float32)
    xf = np.asarray(x, dtype=np.float32).transpose(0, 2, 1).reshape(BS * T, C)
    acc = np.zeros((R, RES, C), np.float32)
    for j in range(MAX_N):
        y = s_[:, None] + ii[None, :] * binsz[:, None] \
            + f32(j + 0.5) * binsz[:, None] / n[:, None]
        valid = (y >= f32(-1.0)) & (y <= f32(T)) & (f32(j) < n[:, None])
        yc = np.maximum(y, f32(0.0))
        ylof = np.floor(yc)
        at_end = ylof >= f32(T - 1)
        ylo = np.clip(ylof.astype(np.int64), 0, T - 1)
        yhi = np.clip(ylo + 1, 0, T - 1)
        ly = np.where(at_end, f32(0.0), yc - ylof).astype(np.float32)
        base = b[:, None] * T
        v = (1.0 - ly)[..., None] * xf[base + ylo] + ly[..., None] * xf[base + yhi]
        acc += np.where(valid[..., None], v, 0.0)
    acc /= n[:, None, None]
    feats = acc.transpose(0, 2, 1).reshape(BS, T, DUR, C * RES)
    return np.ascontiguousarray(feats.transpose(0, 3, 2, 1))


def kernel(x: np.ndarray, anchors: np.ndarray) -> np.ndarray:
    x = np.asarray(x)
    anchors = np.asarray(anchors)
    if not _anchors_batch_uniform(np.asarray(anchors, dtype=np.float32)):
        return _host_fallback(x, anchors)
    Mfull = _build_weight_matrix(anchors)               # [T, RES, DUR, T] f64
    xt16 = np.ascontiguousarray(
        x.astype(np.float32).transpose(0, 2, 1)).astype(np.float16)

    if "nc" not in _CACHE:
        _CACHE["nc"] = _build_nc()
    nc = _CACHE["nc"]

    in_maps = []
    for m in range(N_CORES):
        Mc = np.ascontiguousarray(
            Mfull[:, :, m * DI_PER:(m + 1) * DI_PER, :]
        ).reshape(T, NCOL).astype(np.float16)
        in_maps.append({
            "xt": xt16,
            "mh": np.ascontiguousarray(Mc[0:KA]),
            "ml": np.ascontiguousarray(Mc[KA:T]),
        })

    res = run_bass_kernel_spmd(nc, in_maps, core_ids=list(range(N_CORES)))
    full = np.empty((BS, C, RES, DUR, T), dtype=np.float32)
    for m in range(N_CORES):
        full[:, :, :, m * DI_PER:(m + 1) * DI_PER, :] = \
            res.results[m]["out"].reshape(BS, C, RES, DI_PER, T)
    return full.reshape(BS, C * RES, DUR, T)
